# revision 6
# baseline (speedup 1.0000x reference)
"""Trainium2 Bass kernel for nn_ChimeraV2Block (dual-softmax differential
sliding-window attention block, B=1 S=2048 D=2048, 16 q-heads / 4 kv-heads,
head_dim 128, window 512).

Sharding: tensor-parallel over heads across 8 NeuronCores. Core c owns
q-heads {2c, 2c+1} and kv-head c//2 (GQA groups align with the split).
Wq/Wk/Wv column-sharded, Wo row-sharded; the 8 fp32 partial outputs are
summed on the host (the "all-reduce").

v2 layout notes:
- RoPE runs on bf16 SBUF copies of the projection PSUM (scalar engine
  casts, DVE 2-byte ops at 2x rate).
- gn / v transposes go through the DMA XBAR (dma_start transpose=True)
  instead of PE transpose + PSUM evacuation copies.
- g0 runs on GpSimd; the per-row scalar chain is batched across the two
  heads ([128,2] tiles).
"""

import sys

if "/opt/trn_rl_repo" not in sys.path:
    sys.path.insert(0, "/opt/trn_rl_repo")

import numpy as np
import ml_dtypes

BF = ml_dtypes.bfloat16

S = 2048
D = 2048
H = 16
HK = 4
HD = 128
WIN = 512
THETA = 10000.0
N_CORES = 8
NQT = S // 128          # 16 q row-tiles
NKT = D // 128          # 16 contraction tiles for the projections
WMAX = WIN + 128        # 640: max key-window width per q-tile
NEG = -1.0e30

USE_DMA_T = True        # XBAR transposes for v and gn
USE_GP = True           # g0 on gpsimd

_CACHE = {}


def _tables():
    """RoPE tables [128, S] fp16 with head-dim-duplicated frequencies
    (row p uses invf[p % 64]). The sin table has the rotate-half sign
    folded in and lives at the partition of the SOURCE operand: rows
    64:128 carry -sin (read together with ps[64:128] to produce the low
    output half), rows 0:64 carry +sin. Q tables are pre-scaled by the
    attention scale 1/sqrt(64)."""
    invf = 1.0 / (THETA ** (np.arange(0, HD, 2, dtype=np.float64) / HD))  # [64]
    t = np.arange(S, dtype=np.float64)
    fr = np.outer(invf, t)  # [64, S]
    cosf = np.concatenate([np.cos(fr)] * 2, axis=0)
    sinf = np.concatenate([np.sin(fr), -np.sin(fr)], axis=0)
    return (np.ascontiguousarray(cosf * 0.125, dtype=np.float16),
            np.ascontiguousarray(sinf * 0.125, dtype=np.float16),
            np.ascontiguousarray(cosf, dtype=np.float16),
            np.ascontiguousarray(sinf, dtype=np.float16))


def _masks():
    p = np.arange(128)[:, None]
    c = np.arange(WMAX)[None, :]
    band = (c - p >= 1) & (c - p <= WIN)
    mw = np.where(band, 0.0, NEG).astype(BF)          # [128, 640]
    cc = np.arange(128)[None, :]
    mc = np.where(cc <= p, 0.0, NEG).astype(BF)       # [128, 128] causal
    # edge mask: cols [0,512) allowed, cols [512,640) causal triangle.
    # slicing the last w cols gives the mask for edge q-tiles (qi < 4).
    me = np.zeros((128, WMAX), dtype=BF)
    me[:, WIN:] = mc
    return mw, me


def _build_program():
    import concourse.bacc as bacc
    import concourse.tile as tile
    from concourse import mybir

    bf = mybir.dt.bfloat16
    f32 = mybir.dt.float32
    f16 = mybir.dt.float16
    EXP = mybir.ActivationFunctionType.Exp
    RELU = mybir.ActivationFunctionType.Relu
    MULT = mybir.AluOpType.mult
    ADD = mybir.AluOpType.add
    MAX = mybir.AluOpType.max

    nc = bacc.Bacc("TRN2", target_bir_lowering=False, debug=False,
                   num_devices=N_CORES)

    xt_d = nc.dram_tensor("xt", [128, NKT, S], bf, kind="ExternalInput")
    wq_d = nc.dram_tensor("wq", [128, NKT, 2, 128], bf, kind="ExternalInput")
    wk_d = nc.dram_tensor("wk", [128, NKT, 128], bf, kind="ExternalInput")
    wv_d = nc.dram_tensor("wv", [128, NKT, 128], bf, kind="ExternalInput")
    wo_d = nc.dram_tensor("wo", [128, 2, D], bf, kind="ExternalInput")
    lamn_d = nc.dram_tensor("lamn", [1, 2], f32, kind="ExternalInput")
    out_d = nc.dram_tensor("outp", [S, D], f16, kind="ExternalOutput")

    tqc_np, tqs_np, tkc_np, tks_np = _tables()
    mw_np, me_np = _masks()
    tqc_d = nc.inline_tensor(tqc_np, "tab_qc")
    tqs_d = nc.inline_tensor(tqs_np, "tab_qs")
    tkc_d = nc.inline_tensor(tkc_np, "tab_kc")
    tks_d = nc.inline_tensor(tks_np, "tab_ks")
    mw_d = nc.inline_tensor(mw_np, "mask_win")
    me_d = nc.inline_tensor(me_np, "mask_edge")
    idb_d = nc.inline_tensor(np.eye(128, dtype=BF), "ident_bf")

    with tile.TileContext(nc) as tc:
        with tc.tile_pool(name="xpool", bufs=1) as xp, \
             tc.tile_pool(name="wpool", bufs=1) as wp, \
             tc.tile_pool(name="pers", bufs=1) as pers:

            # DMA issue order matters: projection weights + first x chunk
            # first so matmuls start early; bulk x + tables stream in under
            # compute; phase-2/3 constants (masks, wo) last.
            wq = wp.tile([128, NKT, 2, 128], bf)
            wk = wp.tile([128, NKT, 128], bf)
            nc.sync.dma_start(out=wk[:, 0:8], in_=wk_d[:, 0:8])
            nc.sync.dma_start(out=wk[:, 8:16], in_=wk_d[:, 8:16])
            wv = wp.tile([128, NKT, 128], bf)
            nc.sync.dma_start(out=wv[:, 0:8], in_=wv_d[:, 0:8])
            nc.sync.dma_start(out=wv[:, 8:16], in_=wv_d[:, 8:16])
            idb = wp.tile([128, 128], bf)
            nc.sync.dma_start(out=idb[:], in_=idb_d[:])
            lamn = wp.tile([1, 2], f32)
            nc.sync.dma_start(out=lamn[:], in_=lamn_d[:])

            xts = []
            for nch in range(4):
                xc = xp.tile([128, NKT, 512], bf, tag=f"xt{nch}")
                xts.append(xc)
            tqc = wp.tile([128, S], f16)
            tqs = wp.tile([128, S], f16)
            tkc = wp.tile([128, S], f16)
            tks = wp.tile([128, S], f16)

            def load_chunk(nch, ndma):
                sl = slice(nch * 512, (nch + 1) * 512)
                kstep = NKT // ndma
                for i in range(ndma):
                    ksl = slice(i * kstep, (i + 1) * kstep)
                    nc.sync.dma_start(out=xts[nch][:, ksl, :],
                                      in_=xt_d[:, ksl, sl])

            for i in range(4):
                nc.sync.dma_start(out=wq[:, 4 * i:4 * i + 4],
                                  in_=wq_d[:, 4 * i:4 * i + 4])
                for j in range(2):
                    k2 = slice(4 * i + 2 * j, 4 * i + 2 * (j + 1))
                    nc.sync.dma_start(out=xts[0][:, k2, :],
                                      in_=xt_d[:, k2, 0:512])
            for i in range(2):
                sl = slice(1024 * i, 1024 * (i + 1))
                nc.sync.dma_start(out=tqc[:, sl], in_=tqc_d[:, sl])
                nc.sync.dma_start(out=tqs[:, sl], in_=tqs_d[:, sl])
                nc.sync.dma_start(out=tkc[:, sl], in_=tkc_d[:, sl])
                nc.sync.dma_start(out=tks[:, sl], in_=tks_d[:, sl])
            load_chunk(1, 8)
            load_chunk(2, 4)
            load_chunk(3, 4)
            mw = wp.tile([128, WMAX], bf)
            nc.sync.dma_start(out=mw[:], in_=mw_d[:])
            me = wp.tile([128, WMAX], bf)
            nc.sync.dma_start(out=me[:], in_=me_d[:])
            wo = wp.tile([128, 2, D], bf)
            for i in range(4):
                nc.sync.dma_start(out=wo[:, :, 512 * i:512 * (i + 1)],
                                  in_=wo_d[:, :, 512 * i:512 * (i + 1)])
            lamb = wp.tile([128, 2], f32)
            nc.gpsimd.partition_broadcast(lamb[:], lamn[:])

            # q stored zero-padded to full 128 contraction rows per half:
            # qtp0 rows 0:64 hold half-0 q, rows 64:128 are zero; qtp1 is
            # the mirror. A 128-contraction matmul runs at 2x the column
            # rate of a 64-contraction one, so the padded zeros are free.
            qtp0 = pers.tile([128, 2, S], bf)
            qtp1 = pers.tile([128, 2, S], bf)
            kt = pers.tile([128, S], bf)         # RoPE'd k, hd-major
            vsm = pers.tile([128, NQT, 128], bf)  # v, S-major [s, hd]
            att = pers.tile([128, 2, S], bf)     # attention out^T, hd-major
            nc.gpsimd.memset(qtp0[64:128, :, :], 0.0)
            nc.gpsimd.memset(qtp1[0:64, :, :], 0.0)

            # ---- Phase 1: projections + RoPE + v transpose ----
            with tc.tile_pool(name="pp", bufs=1, space="PSUM") as pp, \
                 tc.tile_pool(name="pt", bufs=2) as pt:
                for nch in range(4):
                    sl = slice(nch * 512, (nch + 1) * 512)
                    ps_q0 = pp.tile([128, 512], f32, tag="pq0", bufs=2)
                    ps_q1 = pp.tile([128, 512], f32, tag="pq1", bufs=2)
                    ps_k = pp.tile([128, 512], f32, tag="pk", bufs=1)
                    ps_v = pp.tile([128, 512], f32, tag="pv", bufs=1)
                    for kti in range(NKT):
                        st = kti == 0
                        sp = kti == NKT - 1
                        rhs = xts[nch][:, kti, :]
                        nc.tensor.matmul(ps_q0[:], wq[:, kti, 0, :], rhs, start=st, stop=sp)
                        nc.tensor.matmul(ps_q1[:], wq[:, kti, 1, :], rhs, start=st, stop=sp)
                        nc.tensor.matmul(ps_k[:], wk[:, kti, :], rhs, start=st, stop=sp)
                        nc.tensor.matmul(ps_v[:], wv[:, kti, :], rhs, start=st, stop=sp)
                    # cast PSUM -> SBUF bf16 on the scalar engine, then all
                    # RoPE math is 2-byte DVE work at 2x rate.
                    for ps, outlo, outhi, tabc, tabs, tg in (
                            (ps_q0, qtp0[0:64, 0, sl], qtp1[64:128, 0, sl], tqc, tqs, "q0"),
                            (ps_q1, qtp0[0:64, 1, sl], qtp1[64:128, 1, sl], tqc, tqs, "q1"),
                            (ps_k, None, None, tkc, tks, "k")):
                        psb = pt.tile([128, 512], bf, tag="psb" + tg)
                        nc.scalar.copy(out=psb[:], in_=ps[:])
                        m1 = pt.tile([128, 512], bf, tag="m1" + tg)
                        m2 = pt.tile([128, 512], bf, tag="m2" + tg)
                        nc.vector.tensor_mul(m1[:], psb[:], tabc[:, sl])
                        nc.vector.tensor_mul(m2[0:64, :], psb[64:128, :], tabs[64:128, sl])
                        nc.vector.tensor_mul(m2[64:128, :], psb[0:64, :], tabs[0:64, sl])
                        if outlo is None:
                            nc.vector.tensor_add(kt[:, sl], m1[:], m2[:])
                        else:
                            nc.vector.tensor_add(outlo, m1[0:64, :], m2[0:64, :])
                            nc.vector.tensor_add(outhi, m1[64:128, :], m2[64:128, :])
                    vtmp = pt.tile([128, 512], bf, tag="vtmp")
                    nc.vector.tensor_copy(out=vtmp[:], in_=ps_v[:])
                    if USE_DMA_T:
                        nc.sync.dma_start(out=vsm[:, 4 * nch:4 * nch + 4, :],
                                          in_=vtmp[:], transpose=True)
                    else:
                        ps_tv = pp.tile([128, 4, 128], bf, tag="ptv", bufs=2)
                        for j in range(4):
                            nc.tensor.transpose(ps_tv[:, j, :], vtmp[:, 128 * j:128 * (j + 1)], idb[:])
                        nc.vector.tensor_copy(out=vsm[:, 4 * nch:4 * (nch + 1), :], in_=ps_tv[:])

            # ---- Phase 2: attention ----
            with tc.tile_pool(name="psc", bufs=1, space="PSUM") as psc, \
                 tc.tile_pool(name="pse", bufs=1) as pse, \
                 tc.tile_pool(name="psm", bufs=1) as psm:
                for qi in range(NQT):
                    qsl = slice(qi * 128, (qi + 1) * 128)
                    kw = min(qi + 1, 5)
                    w = kw * 128
                    kstart = max(0, qi - 4)

                    ps_av = psc.tile([128, 2, 128], f32, tag="av", bufs=1)
                    gts = pse.tile([128, 2, 5, 128], bf, tag="gts", bufs=2)
                    e1s = []
                    e2s = []
                    s1t = psm.tile([128, 2], f32, tag="s1", bufs=2)
                    s2t = psm.tile([128, 2], f32, tag="s2", bufs=2)

                    for h in range(2):
                        ps_s1 = psc.tile([128, WMAX], f32, tag="s", bufs=2)
                        ps_s2 = psc.tile([128, WMAX], f32, tag="s", bufs=2)
                        for ps, lhsq in ((ps_s1, qtp0), (ps_s2, qtp1)):
                            lhs = lhsq[:, h, qsl]
                            kwin = slice(kstart * 128, kstart * 128 + w)
                            if qi >= 4:
                                # causal edge lives only in block 0, so the
                                # mask matmul covers 128 cols; the score
                                # matmul splits at that boundary
                                nc.tensor.matmul(ps[:, 0:128], idb[:], mw[:, 0:128],
                                                 start=True, stop=False)
                                nc.tensor.matmul(ps[:, 0:128], lhs,
                                                 kt[:, kwin][:, 0:128],
                                                 start=False, stop=True)
                                nc.tensor.matmul(ps[:, 128:512], lhs,
                                                 kt[:, kwin][:, 128:512],
                                                 start=True, stop=True)
                                nc.tensor.matmul(ps[:, 512:640], idb[:],
                                                 mw[:, 512:640],
                                                 start=True, stop=False)
                                nc.tensor.matmul(ps[:, 512:640], lhs,
                                                 kt[:, kwin][:, 512:640],
                                                 start=False, stop=True)
                            else:
                                nc.tensor.matmul(ps[:, 0:w], idb[:],
                                                 me[:, WMAX - w:WMAX],
                                                 start=True, stop=False)
                                nc.tensor.matmul(ps[:, 0:w], lhs,
                                                 kt[:, kwin][:, 0:w],
                                                 start=False, stop=True)

                        e1 = pse.tile([128, WMAX], bf, tag="e1", bufs=2)
                        e2 = pse.tile([128, WMAX], bf, tag="e2", bufs=2)
                        nc.scalar.activation(out=e1[:, 0:w], in_=ps_s1[:, 0:w],
                                             func=EXP, accum_out=s1t[:, h:h + 1])
                        nc.scalar.activation(out=e2[:, 0:w], in_=ps_s2[:, 0:w],
                                             func=EXP, accum_out=s2t[:, h:h + 1])
                        e1s.append(e1)
                        e2s.append(e2)

                    # batched per-row scalars for both heads:
                    # cneg = -(lam * s1 / s2)   (lamn holds -lam)
                    r2 = psm.tile([128, 2], f32, tag="r2", bufs=2)
                    nc.vector.reciprocal(out=r2[:], in_=s2t[:])
                    t1 = psm.tile([128, 2], f32, tag="t1", bufs=2)
                    nc.vector.tensor_mul(t1[:], s1t[:], r2[:])
                    cneg = psm.tile([128, 2], f32, tag="cneg", bufs=2)
                    nc.vector.tensor_mul(cneg[:], t1[:], lamb[:])

                    # e2c = cneg*e2 (DVE 2x); g0 = e1 + e2c (gpsimd tt);
                    # g = relu(g0), accum D'
                    dsum = psm.tile([128, 2], f32, tag="dsum", bufs=2)
                    gs = []
                    for h in range(2):
                        e2c = pse.tile([128, WMAX], bf, tag="e2c", bufs=2)
                        nc.vector.tensor_scalar(
                            out=e2c[:, 0:w], in0=e2s[h][:, 0:w],
                            scalar1=cneg[:, h:h + 1], scalar2=0.0,
                            op0=MULT, op1=ADD)
                        g0 = pse.tile([128, WMAX], bf, tag="g0", bufs=2)
                        eng = nc.gpsimd if USE_GP else nc.vector
                        eng.tensor_tensor(out=g0[:, 0:w], in0=e1s[h][:, 0:w],
                                          in1=e2c[:, 0:w], op=ADD)
                        g = pse.tile([128, WMAX], bf, tag="g", bufs=2)
                        if h == 0:
                            nc.scalar.activation(
                                out=g[:, 0:w], in_=g0[:, 0:w], func=RELU,
                                accum_out=dsum[:, 0:1])
                        else:
                            nc.vector.tensor_scalar(
                                out=g[:, 0:w], in0=g0[:, 0:w], scalar1=0.0,
                                scalar2=0.0, op0=MAX, op1=ADD,
                                accum_out=dsum[:, 1:2])
                        gs.append(g)

                    # recd = 1 / (D' + 1e-6 * s1); gn = g * recd
                    dtmp = psm.tile([128, 2], f32, tag="dtmp", bufs=2)
                    nc.vector.scalar_tensor_tensor(
                        out=dtmp[:], in0=s1t[:], scalar=1e-6, in1=dsum[:],
                        op0=MULT, op1=ADD)
                    recd = psm.tile([128, 2], f32, tag="recd", bufs=2)
                    nc.vector.reciprocal(out=recd[:], in_=dtmp[:])

                    for h in range(2):
                        gn = pse.tile([128, WMAX], bf, tag="gn", bufs=2)
                        nc.vector.tensor_scalar(
                            out=gn[:, 0:w], in0=gs[h][:, 0:w],
                            scalar1=recd[:, h:h + 1],
                            scalar2=0.0, op0=MULT, op1=ADD)
                        if USE_DMA_T:
                            nc.sync.dma_start(out=gts[:, h, 0:kw, :],
                                              in_=gn[:, 0:w], transpose=True)
                        else:
                            ps_tr = psc.tile([128, 5, 128], bf, tag="trg", bufs=2)
                            for j in range(kw):
                                nc.tensor.transpose(ps_tr[:, j, :],
                                                    gn[:, 128 * j:128 * (j + 1)], idb[:])
                            if h == 0:
                                nc.vector.tensor_copy(out=gts[:, 0, 0:kw, :], in_=ps_tr[:, 0:kw, :])
                            else:
                                nc.scalar.copy(out=gts[:, 1, 0:kw, :], in_=ps_tr[:, 0:kw, :])

                    # AV for both heads at once: [k,hd]^T-contract x [k, 2*128]
                    for j in range(kw):
                        nc.tensor.matmul(ps_av[:], vsm[:, kstart + j, :],
                                         gts[:, :, j, :],
                                         start=(j == 0), stop=(j == kw - 1))

                    nc.vector.tensor_copy(out=att[:, :, qsl], in_=ps_av[:])

                    # out-projection for this q-tile, interleaved so the PE
                    # fills attention bubbles and the output DMA spreads out
                    so = pse.tile([128, 2048], f16, tag="so", bufs=2)
                    for dch in range(4):
                        dsl = slice(dch * 512, (dch + 1) * 512)
                        ps_o = psc.tile([128, 512], f32, tag="o", bufs=2)
                        nc.tensor.matmul(ps_o[:], att[:, 0, qsl], wo[:, 0, dsl],
                                         start=True, stop=False)
                        nc.tensor.matmul(ps_o[:], att[:, 1, qsl], wo[:, 1, dsl],
                                         start=False, stop=True)
                        if dch % 2 == 0:
                            nc.vector.tensor_copy(out=so[:, dsl], in_=ps_o[:])
                        else:
                            nc.scalar.copy(out=so[:, dsl], in_=ps_o[:])
                        if dch % 2 == 1:
                            dsl2 = slice((dch - 1) * 512, (dch + 1) * 512)
                            nc.sync.dma_start(out=out_d[qsl, dsl2], in_=so[:, dsl2])

    nc.compile()
    return nc


def get_program():
    if "nc" not in _CACHE:
        _CACHE["nc"] = _build_program()
    return _CACHE["nc"]


def _prep_inputs(x, Wq, Wk, Wv, Wo, lam):
    xt = np.ascontiguousarray(x.reshape(S, D).T.astype(BF)
                              .reshape(NKT, 128, S).transpose(1, 0, 2))
    in_maps = []
    for c in range(N_CORES):
        h0 = 2 * c
        kv = c // 2
        wq_c = np.ascontiguousarray(
            Wq[:, h0 * 128:(h0 + 2) * 128].astype(BF)
            .reshape(NKT, 128, 2, 128).transpose(1, 0, 2, 3))
        wk_c = np.ascontiguousarray(
            Wk[:, kv * 128:(kv + 1) * 128].astype(BF)
            .reshape(NKT, 128, 128).transpose(1, 0, 2))
        wv_c = np.ascontiguousarray(
            Wv[:, kv * 128:(kv + 1) * 128].astype(BF)
            .reshape(NKT, 128, 128).transpose(1, 0, 2))
        wo_c = np.ascontiguousarray(
            Wo[h0 * 128:(h0 + 2) * 128, :].astype(BF)
            .reshape(2, 128, D).transpose(1, 0, 2))
        lamn_c = np.array([[-float(lam[h0]), -float(lam[h0 + 1])]], dtype=np.float32)
        in_maps.append({"xt": xt, "wq": wq_c, "wk": wk_c, "wv": wv_c,
                        "wo": wo_c, "lamn": lamn_c})
    return in_maps


def kernel(x, Wq, Wk, Wv, Wo, lam):
    from concourse.bass_utils import run_bass_kernel_spmd

    nc = get_program()
    in_maps = _prep_inputs(np.asarray(x), np.asarray(Wq), np.asarray(Wk),
                           np.asarray(Wv), np.asarray(Wo), np.asarray(lam))
    res = run_bass_kernel_spmd(nc, in_maps, list(range(N_CORES)))
    out = np.zeros((S, D), dtype=np.float32)
    for c in range(N_CORES):
        out += res.results[c]["outp"].astype(np.float32)
    return out.reshape(1, S, D)


# revision 9
# speedup vs baseline: 1.3686x; 1.3686x over previous
"""Trainium2 Bass kernel for nn_ChimeraV2Block (dual-softmax differential
sliding-window attention block, B=1 S=2048 D=2048, 16 q-heads / 4 kv-heads,
head_dim 128, window 512).

Sharding: tensor-parallel over heads across 8 NeuronCores. Core c owns
q-heads {2c, 2c+1} and kv-head c//2 (GQA groups align with the split).
Wq/Wk/Wv column-sharded, Wo row-sharded; the 8 fp32 partial outputs are
summed on the host (the "all-reduce").

v2 layout notes:
- RoPE runs on bf16 SBUF copies of the projection PSUM (scalar engine
  casts, DVE 2-byte ops at 2x rate).
- gn / v transposes go through the DMA XBAR (dma_start transpose=True)
  instead of PE transpose + PSUM evacuation copies.
- g0 runs on GpSimd; the per-row scalar chain is batched across the two
  heads ([128,2] tiles).
"""

import sys

if "/opt/trn_rl_repo" not in sys.path:
    sys.path.insert(0, "/opt/trn_rl_repo")

import numpy as np
import ml_dtypes

BF = ml_dtypes.bfloat16

S = 2048
D = 2048
H = 16
HK = 4
HD = 128
WIN = 512
THETA = 10000.0
N_CORES = 8
NQT = S // 128          # 16 q row-tiles
NKT = D // 128          # 16 contraction tiles for the projections
WMAX = WIN + 128        # 640: max key-window width per q-tile
NEG = -1.0e30

USE_DMA_T = True        # XBAR transposes for v and gn
USE_GP = True           # g0 on gpsimd

_CACHE = {}


def _tables():
    """RoPE tables [128, S] fp16 with head-dim-duplicated frequencies
    (row p uses invf[p % 64]). The sin table has the rotate-half sign
    folded in and lives at the partition of the SOURCE operand: rows
    64:128 carry -sin (read together with ps[64:128] to produce the low
    output half), rows 0:64 carry +sin. Q tables are pre-scaled by the
    attention scale 1/sqrt(64)."""
    invf = 1.0 / (THETA ** (np.arange(0, HD, 2, dtype=np.float64) / HD))  # [64]
    t = np.arange(S, dtype=np.float64)
    fr = np.outer(invf, t)  # [64, S]
    cosf = np.concatenate([np.cos(fr)] * 2, axis=0)
    sinf = np.concatenate([np.sin(fr), -np.sin(fr)], axis=0)
    return (np.ascontiguousarray(cosf * 0.125, dtype=np.float16),
            np.ascontiguousarray(sinf * 0.125, dtype=np.float16),
            np.ascontiguousarray(cosf, dtype=np.float16),
            np.ascontiguousarray(sinf, dtype=np.float16))


def _masks():
    p = np.arange(128)[:, None]
    c = np.arange(WMAX)[None, :]
    band = (c - p >= 1) & (c - p <= WIN)
    mw = np.where(band, 0.0, NEG).astype(BF)          # [128, 640]
    cc = np.arange(128)[None, :]
    mc = np.where(cc <= p, 0.0, NEG).astype(BF)       # [128, 128] causal
    # edge mask: cols [0,512) allowed, cols [512,640) causal triangle.
    # slicing the last w cols gives the mask for edge q-tiles (qi < 4).
    me = np.zeros((128, WMAX), dtype=BF)
    me[:, WIN:] = mc
    return mw, me


def _build_program():
    import concourse.bacc as bacc
    import concourse.tile as tile
    from concourse import mybir

    bf = mybir.dt.bfloat16
    f32 = mybir.dt.float32
    f16 = mybir.dt.float16
    EXP = mybir.ActivationFunctionType.Exp
    RELU = mybir.ActivationFunctionType.Relu
    MULT = mybir.AluOpType.mult
    ADD = mybir.AluOpType.add
    MAX = mybir.AluOpType.max

    nc = bacc.Bacc("TRN2", target_bir_lowering=False, debug=False,
                   num_devices=N_CORES)

    xt_d = nc.dram_tensor("xt", [128, NKT, S], bf, kind="ExternalInput")
    wq_d = nc.dram_tensor("wq", [128, NKT, 2, 128], bf, kind="ExternalInput")
    wk_d = nc.dram_tensor("wk", [128, NKT, 128], bf, kind="ExternalInput")
    wv_d = nc.dram_tensor("wv", [128, NKT, 128], bf, kind="ExternalInput")
    wo_d = nc.dram_tensor("wo", [128, 2, D], bf, kind="ExternalInput")
    lamn_d = nc.dram_tensor("lamn", [1, 2], f32, kind="ExternalInput")
    out_d = nc.dram_tensor("outp", [S, D], f16, kind="ExternalOutput")

    tqc_np, tqs_np, tkc_np, tks_np = _tables()
    mw_np, me_np = _masks()
    tqc_d = nc.inline_tensor(tqc_np, "tab_qc")
    tqs_d = nc.inline_tensor(tqs_np, "tab_qs")
    tkc_d = nc.inline_tensor(tkc_np, "tab_kc")
    tks_d = nc.inline_tensor(tks_np, "tab_ks")
    mw_d = nc.inline_tensor(mw_np, "mask_win")
    me_d = nc.inline_tensor(me_np, "mask_edge")
    idb_d = nc.inline_tensor(np.eye(128, dtype=BF), "ident_bf")

    with tile.TileContext(nc) as tc:
        with tc.tile_pool(name="xpool", bufs=1) as xp, \
             tc.tile_pool(name="wpool", bufs=1) as wp, \
             tc.tile_pool(name="pers", bufs=1) as pers:

            # DMA issue order matters: projection weights + first x chunk
            # first so matmuls start early; bulk x + tables stream in under
            # compute; phase-2/3 constants (masks, wo) last.
            wq = wp.tile([128, NKT, 2, 128], bf)
            wk = wp.tile([128, NKT, 128], bf)
            nc.sync.dma_start(out=wk[:, 0:8], in_=wk_d[:, 0:8])
            nc.sync.dma_start(out=wk[:, 8:16], in_=wk_d[:, 8:16])
            wv = wp.tile([128, NKT, 128], bf)
            nc.sync.dma_start(out=wv[:, 0:8], in_=wv_d[:, 0:8])
            nc.sync.dma_start(out=wv[:, 8:16], in_=wv_d[:, 8:16])
            idb = wp.tile([128, 128], bf)
            nc.sync.dma_start(out=idb[:], in_=idb_d[:])
            lamn = wp.tile([1, 2], f32)
            nc.sync.dma_start(out=lamn[:], in_=lamn_d[:])

            xts = []
            for nch in range(4):
                xc = xp.tile([128, NKT, 512], bf, tag=f"xt{nch}")
                xts.append(xc)
            tqc = wp.tile([128, S], f16)
            tqs = wp.tile([128, S], f16)
            tkc = wp.tile([128, S], f16)
            tks = wp.tile([128, S], f16)

            def load_chunk(nch, ndma):
                sl = slice(nch * 512, (nch + 1) * 512)
                kstep = NKT // ndma
                for i in range(ndma):
                    ksl = slice(i * kstep, (i + 1) * kstep)
                    nc.sync.dma_start(out=xts[nch][:, ksl, :],
                                      in_=xt_d[:, ksl, sl])

            for i in range(4):
                nc.sync.dma_start(out=wq[:, 4 * i:4 * i + 4],
                                  in_=wq_d[:, 4 * i:4 * i + 4])
                for j in range(2):
                    k2 = slice(4 * i + 2 * j, 4 * i + 2 * (j + 1))
                    nc.sync.dma_start(out=xts[0][:, k2, :],
                                      in_=xt_d[:, k2, 0:512])
            for i in range(2):
                sl = slice(1024 * i, 1024 * (i + 1))
                nc.sync.dma_start(out=tqc[:, sl], in_=tqc_d[:, sl])
                nc.sync.dma_start(out=tqs[:, sl], in_=tqs_d[:, sl])
                nc.sync.dma_start(out=tkc[:, sl], in_=tkc_d[:, sl])
                nc.sync.dma_start(out=tks[:, sl], in_=tks_d[:, sl])
            load_chunk(1, 8)
            load_chunk(2, 4)
            load_chunk(3, 4)
            mw = wp.tile([128, WMAX], bf)
            nc.sync.dma_start(out=mw[:], in_=mw_d[:])
            me = wp.tile([128, WMAX], bf)
            nc.sync.dma_start(out=me[:], in_=me_d[:])
            wo = wp.tile([128, 2, D], bf)
            for i in range(4):
                nc.sync.dma_start(out=wo[:, :, 512 * i:512 * (i + 1)],
                                  in_=wo_d[:, :, 512 * i:512 * (i + 1)])
            lamb = wp.tile([128, 2], f32)
            nc.gpsimd.partition_broadcast(lamb[:], lamn[:])

            # q stored zero-padded to full 128 contraction rows per half:
            # qtp0 rows 0:64 hold half-0 q, rows 64:128 are zero; qtp1 is
            # the mirror. A 128-contraction matmul runs at 2x the column
            # rate of a 64-contraction one, so the padded zeros are free.
            qtp0 = pers.tile([128, 2, S], bf)
            qtp1 = pers.tile([128, 2, S], bf)
            kt = pers.tile([128, S], bf)         # RoPE'd k, hd-major
            vsm = pers.tile([128, NQT, 128], bf)  # v, S-major [s, hd]
            att = pers.tile([128, 2, S], bf)     # attention out^T, hd-major
            nc.gpsimd.memset(qtp0[64:128, :, :], 0.0)
            nc.gpsimd.memset(qtp1[0:64, :, :], 0.0)

            # ---- Phase 1: projections + RoPE + v transpose ----
            with tc.tile_pool(name="pp", bufs=1, space="PSUM") as pp, \
                 tc.tile_pool(name="pt", bufs=2) as pt:
                for nch in range(4):
                    sl = slice(nch * 512, (nch + 1) * 512)
                    ps_q0 = pp.tile([128, 512], f32, tag="pq0", bufs=2)
                    ps_q1 = pp.tile([128, 512], f32, tag="pq1", bufs=2)
                    ps_k = pp.tile([128, 512], f32, tag="pk", bufs=1)
                    ps_v = pp.tile([128, 512], f32, tag="pv", bufs=1)
                    for kti in range(NKT):
                        st = kti == 0
                        sp = kti == NKT - 1
                        rhs = xts[nch][:, kti, :]
                        nc.tensor.matmul(ps_q0[:], wq[:, kti, 0, :], rhs, start=st, stop=sp)
                        nc.tensor.matmul(ps_q1[:], wq[:, kti, 1, :], rhs, start=st, stop=sp)
                        nc.tensor.matmul(ps_k[:], wk[:, kti, :], rhs, start=st, stop=sp)
                        nc.tensor.matmul(ps_v[:], wv[:, kti, :], rhs, start=st, stop=sp)
                    # cast PSUM -> SBUF bf16 on the scalar engine, then all
                    # RoPE math is 2-byte DVE work at 2x rate.
                    for ps, outlo, outhi, tabc, tabs, tg in (
                            (ps_q0, qtp0[0:64, 0, sl], qtp1[64:128, 0, sl], tqc, tqs, "q0"),
                            (ps_q1, qtp0[0:64, 1, sl], qtp1[64:128, 1, sl], tqc, tqs, "q1"),
                            (ps_k, None, None, tkc, tks, "k")):
                        psb = pt.tile([128, 512], bf, tag="psb" + tg)
                        nc.scalar.copy(out=psb[:], in_=ps[:])
                        m1 = pt.tile([128, 512], bf, tag="m1" + tg)
                        m2 = pt.tile([128, 512], bf, tag="m2" + tg)
                        nc.vector.tensor_mul(m1[:], psb[:], tabc[:, sl])
                        nc.vector.tensor_mul(m2[0:64, :], psb[64:128, :], tabs[64:128, sl])
                        nc.vector.tensor_mul(m2[64:128, :], psb[0:64, :], tabs[0:64, sl])
                        if outlo is None:
                            nc.vector.tensor_add(kt[:, sl], m1[:], m2[:])
                        else:
                            nc.vector.tensor_add(outlo, m1[0:64, :], m2[0:64, :])
                            nc.vector.tensor_add(outhi, m1[64:128, :], m2[64:128, :])
                    vtmp = pt.tile([128, 512], bf, tag="vtmp")
                    nc.vector.tensor_copy(out=vtmp[:], in_=ps_v[:])
                    if USE_DMA_T:
                        nc.sync.dma_start(out=vsm[:, 4 * nch:4 * nch + 4, :],
                                          in_=vtmp[:], transpose=True)
                    else:
                        ps_tv = pp.tile([128, 4, 128], bf, tag="ptv", bufs=2)
                        for j in range(4):
                            nc.tensor.transpose(ps_tv[:, j, :], vtmp[:, 128 * j:128 * (j + 1)], idb[:])
                        nc.vector.tensor_copy(out=vsm[:, 4 * nch:4 * (nch + 1), :], in_=ps_tv[:])

            # ---- Phase 2: attention ----
            with tc.tile_pool(name="psc", bufs=1, space="PSUM") as psc, \
                 tc.tile_pool(name="pse", bufs=1) as pse, \
                 tc.tile_pool(name="psm", bufs=1) as psm:
                for qi in range(NQT):
                    qsl = slice(qi * 128, (qi + 1) * 128)
                    kw = min(qi + 1, 5)
                    w = kw * 128
                    kstart = max(0, qi - 4)

                    ps_av = psc.tile([128, 2, 128], f32, tag="av", bufs=1)
                    gts = pse.tile([128, 2, 5, 128], bf, tag="gts", bufs=2)
                    e1s = []
                    e2s = []
                    s1t = psm.tile([128, 2], f32, tag="s1", bufs=2)
                    s2t = psm.tile([128, 2], f32, tag="s2", bufs=2)

                    for h in range(2):
                        ps_s1 = psc.tile([128, WMAX], f32, tag="s", bufs=2)
                        ps_s2 = psc.tile([128, WMAX], f32, tag="s", bufs=2)
                        for ps, lhsq in ((ps_s1, qtp0), (ps_s2, qtp1)):
                            lhs = lhsq[:, h, qsl]
                            kwin = slice(kstart * 128, kstart * 128 + w)
                            if qi >= 4:
                                # causal edge lives only in block 0, so the
                                # mask matmul covers 128 cols; the score
                                # matmul splits at that boundary
                                nc.tensor.matmul(ps[:, 0:128], idb[:], mw[:, 0:128],
                                                 start=True, stop=False)
                                nc.tensor.matmul(ps[:, 0:128], lhs,
                                                 kt[:, kwin][:, 0:128],
                                                 start=False, stop=True)
                                nc.tensor.matmul(ps[:, 128:512], lhs,
                                                 kt[:, kwin][:, 128:512],
                                                 start=True, stop=True)
                                nc.tensor.matmul(ps[:, 512:640], idb[:],
                                                 mw[:, 512:640],
                                                 start=True, stop=False)
                                nc.tensor.matmul(ps[:, 512:640], lhs,
                                                 kt[:, kwin][:, 512:640],
                                                 start=False, stop=True)
                            else:
                                nc.tensor.matmul(ps[:, 0:w], idb[:],
                                                 me[:, WMAX - w:WMAX],
                                                 start=True, stop=False)
                                nc.tensor.matmul(ps[:, 0:w], lhs,
                                                 kt[:, kwin][:, 0:w],
                                                 start=False, stop=True)

                        e1 = pse.tile([128, WMAX], bf, tag="e1", bufs=2)
                        e2 = pse.tile([128, WMAX], bf, tag="e2", bufs=2)
                        nc.scalar.activation(out=e1[:, 0:w], in_=ps_s1[:, 0:w],
                                             func=EXP, accum_out=s1t[:, h:h + 1])
                        nc.scalar.activation(out=e2[:, 0:w], in_=ps_s2[:, 0:w],
                                             func=EXP, accum_out=s2t[:, h:h + 1])
                        e1s.append(e1)
                        e2s.append(e2)

                    # batched per-row scalars for both heads:
                    # cneg = -(lam * s1 / s2)   (lamn holds -lam)
                    r2 = psm.tile([128, 2], f32, tag="r2", bufs=2)
                    nc.vector.reciprocal(out=r2[:], in_=s2t[:])
                    t1 = psm.tile([128, 2], f32, tag="t1", bufs=2)
                    nc.vector.tensor_mul(t1[:], s1t[:], r2[:])
                    cneg = psm.tile([128, 2], f32, tag="cneg", bufs=2)
                    nc.vector.tensor_mul(cneg[:], t1[:], lamb[:])

                    # g0 = e1 + cneg*e2 ; g = relu(g0), accum D'
                    dsum = psm.tile([128, 2], f32, tag="dsum", bufs=2)
                    gs = []
                    for h in range(2):
                        g0 = pse.tile([128, WMAX], bf, tag="g0", bufs=2)
                        nc.vector.scalar_tensor_tensor(
                            out=g0[:, 0:w], in0=e2s[h][:, 0:w],
                            scalar=cneg[:, h:h + 1],
                            in1=e1s[h][:, 0:w], op0=MULT, op1=ADD)
                        g = pse.tile([128, WMAX], bf, tag="g", bufs=2)
                        if h == 0:
                            nc.scalar.activation(
                                out=g[:, 0:w], in_=g0[:, 0:w], func=RELU,
                                accum_out=dsum[:, 0:1])
                        else:
                            nc.vector.tensor_scalar(
                                out=g[:, 0:w], in0=g0[:, 0:w], scalar1=0.0,
                                scalar2=0.0, op0=MAX, op1=ADD,
                                accum_out=dsum[:, 1:2])
                        gs.append(g)

                    # recd = 1 / (D' + 1e-6 * s1); gn = g * recd
                    dtmp = psm.tile([128, 2], f32, tag="dtmp", bufs=2)
                    nc.vector.scalar_tensor_tensor(
                        out=dtmp[:], in0=s1t[:], scalar=1e-6, in1=dsum[:],
                        op0=MULT, op1=ADD)
                    recd = psm.tile([128, 2], f32, tag="recd", bufs=2)
                    nc.vector.reciprocal(out=recd[:], in_=dtmp[:])

                    for h in range(2):
                        gn = pse.tile([128, WMAX], bf, tag="gn", bufs=2)
                        nc.vector.tensor_scalar(
                            out=gn[:, 0:w], in0=gs[h][:, 0:w],
                            scalar1=recd[:, h:h + 1],
                            scalar2=0.0, op0=MULT, op1=ADD)
                        ps_tr = psc.tile([128, 5, 128], bf, tag="trg", bufs=2)
                        for j in range(kw):
                            nc.tensor.transpose(ps_tr[:, j, :],
                                                gn[:, 128 * j:128 * (j + 1)], idb[:])
                        if h == 0:
                            nc.vector.tensor_copy(out=gts[:, 0, 0:kw, :], in_=ps_tr[:, 0:kw, :])
                        else:
                            nc.scalar.copy(out=gts[:, 1, 0:kw, :], in_=ps_tr[:, 0:kw, :])

                    # AV for both heads at once: [k,hd]^T-contract x [k, 2*128]
                    for j in range(kw):
                        nc.tensor.matmul(ps_av[:], vsm[:, kstart + j, :],
                                         gts[:, :, j, :],
                                         start=(j == 0), stop=(j == kw - 1))

                    nc.vector.tensor_copy(out=att[:, :, qsl], in_=ps_av[:])

                    # out-projection for this q-tile, interleaved so the PE
                    # fills attention bubbles and the output DMA spreads out
                    so = pse.tile([128, 2048], f16, tag="so", bufs=2)
                    for dch in range(4):
                        dsl = slice(dch * 512, (dch + 1) * 512)
                        ps_o = psc.tile([128, 512], f32, tag="o", bufs=1)
                        nc.tensor.matmul(ps_o[:], att[:, 0, qsl], wo[:, 0, dsl],
                                         start=True, stop=False)
                        nc.tensor.matmul(ps_o[:], att[:, 1, qsl], wo[:, 1, dsl],
                                         start=False, stop=True)
                        if dch % 2 == 0:
                            nc.vector.tensor_copy(out=so[:, dsl], in_=ps_o[:])
                        else:
                            nc.scalar.copy(out=so[:, dsl], in_=ps_o[:])
                        if dch % 2 == 1:
                            dsl2 = slice((dch - 1) * 512, (dch + 1) * 512)
                            nc.sync.dma_start(out=out_d[qsl, dsl2], in_=so[:, dsl2])

    nc.compile()
    return nc


def get_program():
    if "nc" not in _CACHE:
        _CACHE["nc"] = _build_program()
    return _CACHE["nc"]


def _prep_inputs(x, Wq, Wk, Wv, Wo, lam):
    xt = np.ascontiguousarray(x.reshape(S, D).T.astype(BF)
                              .reshape(NKT, 128, S).transpose(1, 0, 2))
    in_maps = []
    for c in range(N_CORES):
        h0 = 2 * c
        kv = c // 2
        wq_c = np.ascontiguousarray(
            Wq[:, h0 * 128:(h0 + 2) * 128].astype(BF)
            .reshape(NKT, 128, 2, 128).transpose(1, 0, 2, 3))
        wk_c = np.ascontiguousarray(
            Wk[:, kv * 128:(kv + 1) * 128].astype(BF)
            .reshape(NKT, 128, 128).transpose(1, 0, 2))
        wv_c = np.ascontiguousarray(
            Wv[:, kv * 128:(kv + 1) * 128].astype(BF)
            .reshape(NKT, 128, 128).transpose(1, 0, 2))
        wo_c = np.ascontiguousarray(
            Wo[h0 * 128:(h0 + 2) * 128, :].astype(BF)
            .reshape(2, 128, D).transpose(1, 0, 2))
        lamn_c = np.array([[-float(lam[h0]), -float(lam[h0 + 1])]], dtype=np.float32)
        in_maps.append({"xt": xt, "wq": wq_c, "wk": wk_c, "wv": wv_c,
                        "wo": wo_c, "lamn": lamn_c})
    return in_maps


def kernel(x, Wq, Wk, Wv, Wo, lam):
    from concourse.bass_utils import run_bass_kernel_spmd

    nc = get_program()
    in_maps = _prep_inputs(np.asarray(x), np.asarray(Wq), np.asarray(Wk),
                           np.asarray(Wv), np.asarray(Wo), np.asarray(lam))
    res = run_bass_kernel_spmd(nc, in_maps, list(range(N_CORES)))
    out = np.zeros((S, D), dtype=np.float32)
    for c in range(N_CORES):
        out += res.results[c]["outp"].astype(np.float32)
    return out.reshape(1, S, D)
